# revision 24
# baseline (speedup 1.0000x reference)
"""Trainium2 Bass kernel for nn_BidirectionalMamba.

Self-contained: hardcodes shapes from the problem spec.

Sharding (8 cores): core = dir*4 + batch*2 + dhalf
  - dir   in {0,1}: forward chain (blocks 0,1) / backward chain (blocks 2,3,
            fed time-reversed input, un-reversed on host)
  - batch in {0,1}
  - dhalf in {0,1}: each core owns 256 of 512 d_inner channels for the scan
            path; stage-0 (LN/in-proj/conv/rank projections) is replicated
            within the pair. One AllReduce (pair) per block reduces the
            output-projection partials.

The program is identical on all 8 cores (SPMD); per-core behavior comes only
from the input data: the host permutes the d_inner channel axis so each
core's OWN 256 channels always occupy d-tiles 0..1, slices the scan-path
weights to the own half, and zero-pads the merge weights so each core
contributes exactly its 128-row share.

On-device layout is d-major [channel, time]. The scan uses
partitions = (8 channels x 16 states), free = time, with
nc.vector.tensor_tensor_scan computing h_t = dA_t * h_{t-1} + dBx_t.
dA args and (delta*xc) replication are built by block-diagonal matmuls on
the tensor engine; the n-reduction y = sum_n C*h is a 0/1-selector matmul.
"""

import numpy as np

B_, S_, DM, DI, N_, R_, K_ = 2, 2048, 256, 512, 16, 32, 4
DH = DI // 2            # 256 own channels per core
NB = 2                  # blocks per chain (per core)
TCk = 512               # time chunk
NCH = S_ // TCk         # 4
NT = DH // 8            # 32 scan tiles (8 ch x 16 states each)
NG = DH // 128          # 2 own 128-channel groups
NXT = DM // 128         # 2 x-tiles
NDT = DI // 128         # 4 full-d tiles

_BUILD_CACHE = {}
import os
_USE_COLLECTIVE = os.environ.get("NO_CC", "") == ""


# ---------------------------------------------------------------- host prep

def _host_inputs(inputs):
    x = np.ascontiguousarray(inputs['x'], dtype=np.float32)        # [B,S,DM]
    in_maps = []
    for core in range(8):
        d = core // 4          # dir
        b = (core // 2) % 2    # batch
        dh = core % 2          # d-half
        # channel permutation: own 256 first
        perm = np.concatenate([np.arange(dh * DH, (dh + 1) * DH),
                               np.arange((1 - dh) * DH, (2 - dh) * DH)])
        m = {}
        xb = x[b] if d == 0 else x[b, ::-1]
        m['x_in'] = np.ascontiguousarray(xb.T).reshape(NXT, 128, S_)

        for i in range(NB):
            g = 2 * d + i      # global block index
            ln_g = inputs['ln_g'][g].astype(np.float32)
            ln_b = inputs['ln_b'][g].astype(np.float32)
            in_w = inputs['in_w'][g].astype(np.float32)            # [DM, 2DI]
            w_scaled = ln_g[:, None] * in_w
            bvec = ln_b @ in_w                                     # [2DI]
            # columns: xs (512, permuted own-first) + z own (256)
            cols = np.concatenate([perm, DI + perm[:DH]])
            wsel = w_scaled[:, cols]                               # [256, 768]
            bsel = bvec[cols]                                      # [768]
            m[f'inw_{i}'] = np.ascontiguousarray(wsel.reshape(2, 128, 768))
            m[f'bvec_{i}'] = np.ascontiguousarray(bsel.reshape(6, 128, 1))
            cw = inputs['conv_w'][g].astype(np.float32)[perm]      # [DI, K]
            m[f'convw_{i}'] = np.ascontiguousarray(cw.reshape(NDT, 128, K_))
            m[f'convb_{i}'] = np.ascontiguousarray(
                inputs['conv_b'][g].astype(np.float32)[perm].reshape(NDT, 128, 1))
            xp = np.concatenate([inputs['xd_w'][g], inputs['xB_w'][g],
                                 inputs['xC_w'][g]], axis=1)[perm]  # [DI, 64]
            m[f'xproj_{i}'] = np.ascontiguousarray(
                xp.astype(np.float32).reshape(NDT, 128, R_ + 2 * N_))
            own = slice(dh * DH, (dh + 1) * DH)
            m[f'dtpw_{i}'] = np.ascontiguousarray(
                inputs['dtp_w'][g].astype(np.float32)[:, own])      # [32, 256]
            m[f'dtpb_{i}'] = np.ascontiguousarray(
                inputs['dtp_b'][g][own].astype(np.float32).reshape(NG, 128, 1))
            A = -np.exp(inputs['A_log'][g].astype(np.float64)).astype(np.float32)
            Ao = A[own]                                             # [256, N]
            # lhsT packs for K=64 windows: tile k (g=k//16, j=k%16) sits at
            # partition base W=64*(j//8), col-block cb=g*8+(j%8); within the
            # window, lane di maps to row 8*(j%8)+di.
            ablk = np.zeros((128, 16 * 128), np.float32)
            for k in range(NT):
                gg, j = k // 16, k % 16
                Wb, r = 64 * (j // 8), j % 8
                cb = gg * 8 + r
                for di in range(8):
                    ablk[Wb + 8 * r + di,
                         cb * 128 + di * N_:cb * 128 + di * N_ + N_] = Ao[8 * k + di]
            m[f'ablk_{i}'] = np.ascontiguousarray(ablk)
            m[f'dvec_{i}'] = np.ascontiguousarray(
                inputs['D'][g][own].astype(np.float32).reshape(NG, 128, 1))
            ow = inputs['out_w'][g].astype(np.float32)[own]         # [256, 256]
            m[f'outw_{i}'] = np.ascontiguousarray(ow.reshape(2, 128, DM))

        mw = inputs['merge_w'].astype(np.float32)                   # [512, 256]
        mwb = np.zeros((2, 128, DM), np.float32)
        mwb[dh] = mw[d * 256 + dh * 128: d * 256 + (dh + 1) * 128, :]
        m['mergew'] = mwb

        # replication lhsT: window base 64q, col-block r; row 8r+di -> all
        # (di,n) lanes of that tile
        ones_blk = np.zeros((128, 8 * 128), np.float32)
        for q in range(2):
            for r in range(8):
                for di in range(8):
                    ones_blk[64 * q + 8 * r + di,
                             r * 128 + di * N_: r * 128 + (di + 1) * N_] = 1.0
        m['onesblk'] = ones_blk
        # selector for y-reduce into 64-row psum groups: variant r maps lane
        # (di,n) -> group row 8r+di
        sel = np.zeros((128, 8 * 64), np.float32)
        for r in range(8):
            for di in range(8):
                sel[di * N_:(di + 1) * N_, r * 64 + 8 * r + di] = 1.0
        m['sel'] = sel
        m['ones128'] = np.full((128, 1), 1.0 / DM, np.float32)
        cb = np.zeros((128, 2), np.float32)
        cb[:, 0] = 1e-5
        cb[:, 1] = 1.0
        m['cbias'] = cb
        m['ones1'] = np.ones((1, 128), np.float32)
        in_maps.append(m)
    return in_maps


# ---------------------------------------------------------------- device

def _build_program():
    from contextlib import ExitStack
    import concourse.bass as bass
    import concourse.tile as tile
    from concourse import bacc, mybir

    f32 = mybir.dt.float32

    nc = bacc.Bacc("TRN2", target_bir_lowering=False, debug=False,
                   num_devices=8)

    def din(name, shape):
        return nc.dram_tensor(name, list(shape), f32, kind="ExternalInput").ap()

    x_in = din('x_in', (NXT, 128, S_))
    W = {'x_in': x_in}
    for i in range(NB):
        W[f'inw_{i}'] = din(f'inw_{i}', (2, 128, 768))
        W[f'bvec_{i}'] = din(f'bvec_{i}', (6, 128, 1))
        W[f'convw_{i}'] = din(f'convw_{i}', (NDT, 128, K_))
        W[f'convb_{i}'] = din(f'convb_{i}', (NDT, 128, 1))
        W[f'xproj_{i}'] = din(f'xproj_{i}', (NDT, 128, R_ + 2 * N_))
        W[f'dtpw_{i}'] = din(f'dtpw_{i}', (R_, DH))
        W[f'dtpb_{i}'] = din(f'dtpb_{i}', (NG, 128, 1))
        W[f'ablk_{i}'] = din(f'ablk_{i}', (128, 16 * 128))
        W[f'dvec_{i}'] = din(f'dvec_{i}', (NG, 128, 1))
        W[f'outw_{i}'] = din(f'outw_{i}', (2, 128, DM))
    W['mergew'] = din('mergew', (2, 128, DM))
    W['onesblk'] = din('onesblk', (128, 8 * 128))
    W['sel'] = din('sel', (128, 8 * 64))
    W['ones128'] = din('ones128', (128, 1))
    W['cbias'] = din('cbias', (128, 2))
    W['ones1'] = din('ones1', (1, 128))

    outp = nc.dram_tensor('outp', [DM, S_], f32, kind="ExternalOutput").ap()

    with tile.TileContext(nc) as tc:
        with ExitStack() as ctx:
            _emit(ctx, nc, tc, bass, mybir, f32, W, outp)
    nc.compile()
    return nc


def _emit(ctx, nc, tc, bass, mybir, f32, W, outp):
    EX = ctx.enter_context
    AF = mybir.ActivationFunctionType
    OPa, OPm = mybir.AluOpType.add, mybir.AluOpType.mult
    ts = bass.ts

    # ---- pools (sizes noted as KB/partition)
    wpool = EX(tc.tile_pool(name="wconst", bufs=1))
    wblk = EX(tc.tile_pool(name="wblk", bufs=1))
    xio = EX(tc.tile_pool(name="xio", bufs=2))          # xT tags: 2x2x8K = 32K
    lnp = EX(tc.tile_pool(name="lnp", bufs=1))          # rows + reps
    stg = EX(tc.tile_pool(name="stg", bufs=2))          # sq/xn/sptmp chunks
    xsp = EX(tc.tile_pool(name="xsp", bufs=2))          # xs halo chunks 4x2x2K
    xcp = EX(tc.tile_pool(name="xcp", bufs=2))          # xc chunks 4x3x2K
    zsp = EX(tc.tile_pool(name="zsp", bufs=2))
    dlp = EX(tc.tile_pool(name="dlp", bufs=2))
    dxp = EX(tc.tile_pool(name="dxp", bufs=2))
    rkp = EX(tc.tile_pool(name="rkp", bufs=2))
    btp = EX(tc.tile_pool(name="btp", bufs=2))
    scp = EX(tc.tile_pool(name="scp", bufs=2))          # dA/dBx/hC
    hp = EX(tc.tile_pool(name="hp", bufs=2))
    cyp = EX(tc.tile_pool(name="cyp", bufs=NT + 6))     # carry [128,1]
    pop = EX(tc.tile_pool(name="pop", bufs=2))          # ygp/op/ar

    ps_stage = EX(tc.tile_pool(name="ps_stage", bufs=2, space="PSUM"))
    ps_arg = EX(tc.tile_pool(name="ps_arg", bufs=2, space="PSUM"))
    ps_dxr = EX(tc.tile_pool(name="ps_dxr", bufs=2, space="PSUM"))
    ps_y = EX(tc.tile_pool(name="ps_y", bufs=1, space="PSUM"))
    dram = EX(tc.tile_pool(name="dram", bufs=3, space="DRAM"))

    # ---- constants
    def cw(name, shape):
        t = wpool.tile(list(shape), f32, tag=name)
        nc.sync.dma_start(t[:], W[name][:])
        return t

    onesblk = cw('onesblk', (128, 8 * 128))
    sel = cw('sel', (128, 8 * 64))
    ones128 = cw('ones128', (128, 1))
    cbias = cw('cbias', (128, 2))
    ones1 = cw('ones1', (1, 128))
    mergew = [wpool.tile([128, DM], f32, tag=f"mgw{j}", name=f"mgw{j}")
              for j in range(2)]
    for j in range(2):
        nc.sync.dma_start(mergew[j][:], W['mergew'][j])

    # ---- x input
    xT = [xio.tile([128, S_], f32, tag=f"xT{j}", name=f"xTin{j}")
          for j in range(NXT)]
    for j in range(NXT):
        nc.sync.dma_start(xT[j][:], W['x_in'][j])

    replica_groups = [[0, 1], [2, 3], [4, 5], [6, 7]]

    for blk in range(NB):
        # ---- per-block weights
        w = {}
        for nm, cnt, shp in [('inw', 2, (128, 768)), ('bvec', 6, (128, 1)),
                             ('convw', NDT, (128, K_)), ('convb', NDT, (128, 1)),
                             ('xproj', NDT, (128, R_ + 2 * N_)),
                             ('dtpb', NG, (128, 1)), ('dvec', NG, (128, 1)),
                             ('outw', 2, (128, DM))]:
            tl = []
            for j in range(cnt):
                t = wblk.tile(list(shp), f32, tag=f"{nm}{j}")
                nc.sync.dma_start(t[:], W[f'{nm}_{blk}'][j])
                tl.append(t)
            w[nm] = tl
        w['dtpw'] = wblk.tile([R_, DH], f32, tag="dtpw", name=f"dtpw{blk}")
        nc.sync.dma_start(w['dtpw'][:], W[f'dtpw_{blk}'][:])
        w['ablk'] = wblk.tile([128, 16 * 128], f32, tag="ablk", name=f"ablk{blk}", bufs=1)
        nc.sync.dma_start(w['ablk'][:], W[f'ablk_{blk}'][:])

        # ============================== LN stats (pre-loop)
        sumx = lnp.tile([1, S_], f32, tag="sumx")
        sumsq = lnp.tile([1, S_], f32, tag="sumsq")
        for c in range(NCH):
            cs = ts(c, TCk)
            st = ps_stage.tile([128, TCk], f32, tag="ps", name=f"st{c}")
            st2 = ps_stage.tile([128, TCk], f32, tag="ps", name=f"st2{c}")
            sq = [stg.tile([128, TCk], f32, tag="sq", bufs=3, name=f"sq{j}_{c}")
                  for j in range(NXT)]
            for j in range(NXT):
                nc.scalar.activation(sq[j][:], xT[j][:, cs], AF.Square)
            for j in range(NXT):
                nc.tensor.matmul(st[0:1, :], ones128[:], xT[j][:, cs],
                                 start=(j == 0), stop=(j == NXT - 1))
            for j in range(NXT):
                nc.tensor.matmul(st2[0:1, :], ones128[:], sq[j][:],
                                 start=(j == 0), stop=(j == NXT - 1))
            nc.scalar.copy(sumx[0:1, cs], st[0:1, :])
            nc.scalar.copy(sumsq[0:1, cs], st2[0:1, :])
            var = lnp.tile([1, TCk], f32, tag="var", bufs=2, name=f"var{c}")
            nc.vector.tensor_mul(var[:], sumx[0:1, cs], sumx[0:1, cs])
            nc.vector.tensor_sub(var[:], sumsq[0:1, cs], var[:])
            nc.scalar.activation(var[:], var[:], AF.Ln, bias=cbias[0:1, 0:1])
            nc.scalar.activation(sumsq[0:1, cs], var[:], AF.Exp, scale=-0.5)

        # ============================== pipelined chunk loop
        xs_prev = [None] * NDT
        carry = [None] * NT
        xT_next = [xio.tile([128, S_], f32, tag=f"xT{j}", name=f"xTn{blk}_{j}")
                   for j in range(NXT)]
        for c in range(NCH):
            cs = ts(c, TCk)
            # ---- LN normalize
            mrep = ps_stage.tile([128, TCk], f32, tag="ps", name=f"mrep{c}")
            rrep = ps_stage.tile([128, TCk], f32, tag="ps", name=f"rrep{c}")
            nc.tensor.matmul(mrep[:], ones1[:], sumx[0:1, cs],
                             start=True, stop=True)
            nc.tensor.matmul(rrep[:], ones1[:], sumsq[0:1, cs],
                             start=True, stop=True)
            xn = []
            for j in range(NXT):
                t = stg.tile([128, TCk], f32, tag=f"xn{j}")
                nc.vector.tensor_sub(t[:], xT[j][:, cs], mrep[:])
                nc.vector.tensor_mul(t[:], t[:], rrep[:])
                xn.append(t)
            # ---- in-proj (m 0..3 xs d-tiles, 4..5 z own groups), conv, silu
            xc_t = [None] * NDT
            zs_t = [None] * NG
            for m in range(6):
                ps = ps_stage.tile([128, TCk], f32, tag="ps", name=f"pi{c}_{m}")
                for k in range(2):
                    nc.tensor.matmul(ps[:], w['inw'][k][:, ts(m, 128)],
                                     xn[k][:], start=(k == 0), stop=(k == 1))
                if m < NDT:
                    xt = xsp.tile([128, TCk + 4], f32, tag=f"xs{m}")
                    nc.scalar.activation(xt[:, 4:], ps[:], AF.Identity,
                                         bias=w['bvec'][m][:])
                    if c == 0:
                        nc.vector.memset(xt[:, 0:4], 0.0)
                    else:
                        nc.vector.tensor_copy(
                            xt[:, 1:4], xs_prev[m][:, TCk + 1:TCk + 4])
                    xc = xcp.tile([128, TCk], f32, tag=f"xc{m}")
                    nc.vector.tensor_scalar_mul(
                        xc[:], xt[:, 4:], w['convw'][m][:, 3:4])
                    for sh in range(1, K_):
                        nc.vector.scalar_tensor_tensor(
                            xc[:], xt[:, 4 - sh:TCk + 4 - sh],
                            w['convw'][m][:, 3 - sh:4 - sh], xc[:], OPm, OPa)
                    nc.scalar.activation(xc[:], xc[:], AF.Silu,
                                         bias=w['convb'][m][:])
                    xs_prev[m] = xt
                    xc_t[m] = xc
                else:
                    g = m - NDT
                    zt = zsp.tile([128, TCk], f32, tag=f"zs{g}")
                    nc.scalar.activation(zt[:], ps[:], AF.Silu,
                                         bias=w['bvec'][m][:])
                    zs_t[g] = zt
            # ---- rank projections
            psw = ps_stage.tile([128, TCk], f32, tag="ps", name=f"prw{c}")
            psb = ps_stage.tile([128, TCk], f32, tag="ps", name=f"prb{c}")
            psc = ps_stage.tile([128, TCk], f32, tag="ps", name=f"prc{c}")
            for pst, srcoff, width in ((psw, 0, R_), (psb, R_, N_),
                                       (psc, R_ + N_, N_)):
                for k in range(NDT):
                    nc.tensor.matmul(pst[0:width, :],
                                     w['xproj'][k][:, srcoff:srcoff + width],
                                     xc_t[k][:], start=(k == 0),
                                     stop=(k == NDT - 1))
            xcw = rkp.tile([R_, TCk], f32, tag="xcw", bufs=1)
            nc.scalar.copy(xcw[:], psw[0:R_, :])
            bt = rkp.tile([N_, TCk], f32, tag="bt", bufs=1)
            nc.scalar.copy(bt[:], psb[0:N_, :])
            ct = rkp.tile([N_, TCk], f32, tag="ct", bufs=1)
            nc.scalar.copy(ct[:], psc[0:N_, :])
            # ---- delta = softplus(dtpw @ xcw + dtpb), dx = delta * xc_own
            dl_t = [None] * NG
            dx_t = [None] * NG
            for g in range(NG):
                psd = ps_stage.tile([128, TCk], f32, tag="ps", name=f"pd{c}_{g}")
                nc.tensor.matmul(psd[:], w['dtpw'][:, ts(g, 128)], xcw[:],
                                 start=True, stop=True)
                et = stg.tile([128, TCk], f32, tag="sptmp")
                nc.scalar.activation(et[:], psd[:], AF.Exp,
                                     bias=w['dtpb'][g][:])
                dl = dlp.tile([128, TCk], f32, tag=f"dl{g}")
                nc.scalar.activation(dl[:], et[:], AF.Ln, bias=cbias[:, 1:2])
                dxt = dxp.tile([128, TCk], f32, tag=f"dx{g}")
                nc.vector.tensor_mul(dxt[:], dl[:], xc_t[g][:])
                dl_t[g] = dl
                dx_t[g] = dxt
            # ---- B~/C~ replication
            bt2 = btp.tile([128, TCk], f32, tag="btil")
            ct2 = btp.tile([128, TCk], f32, tag="ctil")
            for r in range(8):
                nc.sync.dma_start(bt2[16 * r:16 * r + 16, :], bt[:])
                nc.sync.dma_start(ct2[16 * r:16 * r + 16, :], ct[:])
            # ---- scan sweep
            yps = [ps_y.tile([128, TCk], f32, tag=f"y{g}", name=f"yps{g}_{c}")
                   for g in range(NG)]
            for k in range(NT):
                g, j = k // 16, k % 16
                Wb, r = 64 * (j // 8), j % 8
                win = slice(Wb, Wb + 64)
                psA = ps_arg.tile([128, TCk], f32, tag="psA")
                nc.tensor.matmul(psA[:], w['ablk'][win, ts(g * 8 + r, 128)],
                                 dl_t[g][win, :], start=True, stop=True)
                dA = scp.tile([128, TCk], f32, tag="dA")
                nc.scalar.activation(dA[:], psA[:], AF.Exp)
                psX = ps_dxr.tile([128, TCk], f32, tag="psX")
                nc.tensor.matmul(psX[:], onesblk[win, ts(r, 128)],
                                 dx_t[g][win, :], start=True, stop=True)
                dBx = scp.tile([128, TCk], f32, tag="dBx")
                nc.vector.tensor_mul(dBx[:], psX[:], bt2[:])
                h = hp.tile([128, TCk], f32, tag="h")
                nc.vector.tensor_tensor_scan(
                    h[:], dA[:], dBx[:],
                    0.0 if c == 0 else carry[k][:, 0:1],
                    op0=OPm, op1=OPa)
                if c < NCH - 1:
                    cy = cyp.tile([128, 1], f32, tag="carry")
                    nc.vector.tensor_copy(cy[:], h[:, TCk - 1:TCk])
                    carry[k] = cy
                hC = scp.tile([128, TCk], f32, tag="hC")
                nc.vector.tensor_mul(hC[:], h[:], ct2[:])
                nc.tensor.matmul(yps[g][Wb:Wb + 64, :], sel[:, ts(r, 64)],
                                 hC[:], start=(r == 0), stop=(r == 7))
            # ---- post + out-proj + AllReduce + residual
            yg = []
            for g in range(NG):
                ygp = pop.tile([128, TCk], f32, tag=f"ygp{g}")
                nc.vector.scalar_tensor_tensor(
                    ygp[:], xc_t[g][:], w['dvec'][g][:], yps[g][:],
                    OPm, OPa)
                nc.vector.tensor_mul(ygp[:], ygp[:], zs_t[g][:])
                yg.append(ygp)
            bounce_in = dram.tile([DM, TCk], f32, tag="bin")
            bounce_out = dram.tile([DM, TCk], f32, tag="bout")
            for m in range(NXT):
                ps = ps_stage.tile([128, TCk], f32, tag="ps", name=f"po{c}_{m}")
                for k in range(2):
                    nc.tensor.matmul(ps[:], w['outw'][k][:, ts(m, 128)],
                                     yg[k][:], start=(k == 0), stop=(k == 1))
                op = pop.tile([128, TCk], f32, tag="op", bufs=3)
                nc.scalar.copy(op[:], ps[:])
                nc.sync.dma_start(bounce_in[128 * m:128 * (m + 1), :], op[:])
            if _USE_COLLECTIVE:
                nc.gpsimd.collective_compute(
                    "AllReduce", mybir.AluOpType.add,
                    replica_groups=replica_groups,
                    ins=[bounce_in[:].opt()], outs=[bounce_out[:].opt()])
            else:
                nc.sync.dma_start(bounce_out[:], bounce_in[:])
            for m in range(NXT):
                ar = pop.tile([128, TCk], f32, tag="op", bufs=3, name=f"ar{c}_{m}")
                nc.sync.dma_start(ar[:], bounce_out[128 * m:128 * (m + 1), :])
                nc.vector.tensor_add(xT_next[m][:, cs], ar[:], xT[m][:, cs])
        xT = xT_next

    # ============================== merge partial (zero-padded per core)
    for c in range(NCH):
        cs = ts(c, TCk)
        for m in range(NXT):
            ps = ps_stage.tile([128, TCk], f32, tag="ps")
            for j in range(2):
                nc.tensor.matmul(ps[:], mergew[j][:, ts(m, 128)],
                                 xT[j][:, cs], start=(j == 0), stop=(j == 1))
            op = pop.tile([128, TCk], f32, tag="op", bufs=3)
            nc.scalar.copy(op[:], ps[:])
            nc.sync.dma_start(outp[128 * m:128 * (m + 1), cs], op[:])


# ---------------------------------------------------------------- entry

def kernel(**inputs) -> np.ndarray:
    from concourse.bass_utils import run_bass_kernel_spmd

    if 'nc' not in _BUILD_CACHE:
        _BUILD_CACHE['nc'] = _build_program()
    nc = _BUILD_CACHE['nc']

    in_maps = _host_inputs(inputs)
    res = run_bass_kernel_spmd(nc, in_maps, core_ids=list(range(8)))
    _BUILD_CACHE['last_res'] = res
    parts = [r['outp'] for r in res.results]   # [256, 2048] each

    out = np.zeros((B_, S_, DM), np.float32)
    for b in range(B_):
        acc = np.zeros((DM, S_), np.float32)
        for dh in range(2):
            acc += parts[0 * 4 + b * 2 + dh]               # fwd
            acc += parts[1 * 4 + b * 2 + dh][:, ::-1]      # bwd
        out[b] = acc.T
    return out
